# revision 28
# baseline (speedup 1.0000x reference)
"""Causal self-attention Trainium2 kernel.

B=2, T=2048, C=1024, H=16, D=64. 8 NeuronCores: core = b*4 + head_group,
data parallel over batch (b = core//4), tensor parallel over heads
(4 heads per core). Each core computes its heads' qkv projection,
causal+key-masked attention, and a partial output projection over its
256 input channels; the host sums the per-core/per-head-pair partials
per batch element and adds the proj bias.

All on-device layouts are transposed so softmax works per-partition:
  xT   [C, T]      q^T/k^T [2*64, T] per head-pair (partition = head dim)
  s^T  [k, q]      out^T[d, q]  col-tiled p@v; lhsT of the proj matmul
The softmax denominator l is fused into the pv matmul via [v|ones] /
[ones|v] stationary operands; key padding is data (masked ones in the
l slots + masked v rows). The causal boundary is a -30k DVE bias add on
the PSUM diagonal blocks before exp.

v2 layout notes (DMA issue on the sync engine costs ~0.7us per DMA
regardless of size, so inputs are host-packed into one fused DRAM
tensor per logical matrix and moved with ~12 big DMAs; the l-slot ones
masks are generated on-device from vmask instead of DMA'd):
  x    [128, half, ct, 1024]   w{q,k,v} [128, ct, ot, 128]
  wp   [128, hp, 1024]         va [128, tt, hp, (vh0|ones|ones|vh1)]
"""

import sys

sys.path.insert(0, "/opt/trn_rl_repo")

import numpy as np
import ml_dtypes

import concourse.bass as bass
import concourse.mybir as mybir
import concourse.tile as tile
from concourse import bacc
from concourse.bass import ts, ds
from concourse.bass_utils import run_bass_kernel_spmd

B, T, C, H = 2, 2048, 1024, 16
D = C // H            # 64
HPC = 4               # heads per core
CS = HPC * D          # 256 channel slice per core
NCORE = 8
NKT = T // 128        # 16 k-tiles
NPAIR = NKT // 2      # 8 k-tile pairs
NCH = T // 512        # 4 q-chunks
NCT = C // 128        # 8 contraction tiles
F32 = mybir.dt.float32
BF16 = mybir.dt.bfloat16
EXP = mybir.ActivationFunctionType.Exp

NEG = -30000.0
NVUP = 4              # v chunks drained before attention starts


def build_nc():
    nc = bacc.Bacc("TRN2", target_bir_lowering=False, debug=False,
                   num_devices=NCORE)

    xT = nc.dram_tensor("xT", [128, 2, NCT, 1024], BF16, kind="ExternalInput")
    wq = nc.dram_tensor("wq", [128, NCT, 2, 128], BF16, kind="ExternalInput")
    wk = nc.dram_tensor("wk", [128, NCT, 2, 128], BF16, kind="ExternalInput")
    wv = nc.dram_tensor("wv", [128, NCT, 2, 128], BF16, kind="ExternalInput")
    wp = nc.dram_tensor("wp", [128, 2, 1024], BF16, kind="ExternalInput")
    bqk = nc.dram_tensor("bqk", [128, 4], F32, kind="ExternalInput")
    vbias = nc.dram_tensor("vbias", [128, 2, 2, D], F32, kind="ExternalInput")
    vmask = nc.dram_tensor("vmask", [128, NKT], F32, kind="ExternalInput")
    # trib[p, c] = 0 where c >= p (valid causal), -30000 where c < p;
    # added to the PSUM diagonal score blocks before exp.
    trib = nc.dram_tensor("trib", [128, 128], F32, kind="ExternalInput")
    y = nc.dram_tensor("y", [T, C], BF16, kind="ExternalOutput")

    with tile.TileContext(nc) as tc:
        with (
            tc.tile_pool(name="const", bufs=1) as const,
            tc.tile_pool(name="acts", bufs=1) as acts,
            tc.tile_pool(name="p", bufs=8) as ppool,
            tc.tile_pool(name="ev", bufs=4) as ev,
            tc.tile_pool(name="psum", bufs=1, space="PSUM") as psum,
        ):
            x_t = const.tile([128, 2, NCT, 1024], BF16, tag="x")
            wq_t = const.tile([128, NCT, 2, 128], BF16, tag="wq")
            wk_t = const.tile([128, NCT, 2, 128], BF16, tag="wk")
            wv_t = const.tile([128, NCT, 2, 128], BF16, tag="wv")
            wp_t = const.tile([128, 2, 1024], BF16, tag="wp")
            bqk_t = const.tile([128, 4], F32, tag="bqk")
            vb_t = const.tile([128, 2, 2, D], F32, tag="vb")
            vm_t = const.tile([128, NKT], F32, tag="vm")
            tri_t = const.tile([128, 128], F32, tag="tri")
            # warm-up / ones tile: [128, 5, 128] of 1.0
            warm = const.tile([128, 5, 128], BF16, tag="warm")

            # PE warm-up: dummy matmuls keep the HAM clock warming while
            # the input DMAs stream in; warm also acts as the ones source
            # for the l-slot fills.
            nc.vector.memset(warm[:], 1.0)
            wps = psum.tile([128, 512], F32, tag="pj", name="ps_warm", bufs=2)
            for i in range(10):
                nc.tensor.matmul(wps[:], warm[:, 0, :], warm[:, 1:5, :],
                                 start=True, stop=True, skip_group_check=True)

            # ---- input DMAs: few, big, ordered by first use ----
            nc.sync.dma_start(x_t[:, 0, 0:2, :], xT[:, 0, 0:2, :])
            nc.sync.dma_start(wq_t[:], wq[:])
            for c0 in range(2, NCT, 2):
                nc.sync.dma_start(x_t[:, 0, c0:c0 + 2, :],
                                  xT[:, 0, c0:c0 + 2, :])
            nc.sync.dma_start(wk_t[:], wk[:])
            nc.sync.dma_start(bqk_t[:], bqk[:])
            nc.sync.dma_start(wv_t[:], wv[:])
            nc.sync.dma_start(vb_t[:], vbias[:])
            nc.sync.dma_start(vm_t[:], vmask[:])
            nc.sync.dma_start(tri_t[:], trib[:])
            nc.sync.dma_start(x_t[:, 1, 0:4, :], xT[:, 1, 0:4, :])
            nc.sync.dma_start(x_t[:, 1, 4:8, :], xT[:, 1, 4:8, :])
            nc.sync.dma_start(wp_t[:], wp[:])

            # ---- phase 1: q^T/k^T/v; hp1's q/k and most of v are deferred
            # into attention-hp0's PE stream via generators ----
            qT = [acts.tile([128, T], BF16, tag=f"qT{hp}", name=f"qT{hp}")
                  for hp in range(2)]
            kT = [acts.tile([128, T], BF16, tag=f"kT{hp}", name=f"kT{hp}")
                  for hp in range(2)]
            # va [128 tok, tt, hp, 256]: per (tt, hp) the 256 cols are
            # [v_h0 | ones | ones | v_h1]; head h's 128-wide lhsT slice
            # [128h, 128h+128) is [v|ones] (h=0) / [ones|v] (h=1), so the
            # fused pv+l matmul writes o exactly on the partitions attnT
            # needs (o rows 0-63 for h=0, 64-127 for h=1; the softmax
            # denominator l lands on the other half). The ones slots are
            # vmask-masked so padded keys drop out of the denominator.
            va = acts.tile([128, NKT, 2, 4 * D], BF16, tag="va")

            def xsl(ct, lo, n):
                """x columns [lo, lo+n) of contraction tile ct."""
                return x_t[:, lo // 1024, ct, ds(lo % 1024, n)]

            def qk_chunk_gen(w_t, bcol, dst, ot, tch):
                ps = psum.tile([128, 512], F32, tag="pj", name="ps_qk",
                               bufs=2)
                for ct in range(NCT):
                    nc.tensor.matmul(
                        ps[:],
                        w_t[:, ct, ot, :],
                        xsl(ct, 512 * tch, 512),
                        start=(ct == 0), stop=(ct == NCT - 1),
                        skip_group_check=True,
                    )
                    yield
                nc.vector.tensor_scalar_add(
                    dst[ot][:, ts(tch, 512)], ps[:], bqk_t[:, bcol + ot:bcol + ot + 1])
                yield

            def v_chunk_gen(tt):
                ps = psum.tile([128, 2, 2, D], F32, tag="pj", name="ps_v",
                               bufs=2)
                for ct in range(NCT):
                    nc.tensor.matmul(
                        ps[:],
                        xsl(ct, 128 * tt, 128),
                        wv_t[:, ct, :, :],
                        start=(ct == 0), stop=(ct == NCT - 1),
                        skip_group_check=True,
                    )
                    yield
                # psum [128, (hp, h, d)]; h=0 -> cols 0:64, h=1 -> 192:256
                nc.vector.tensor_add(
                    va[:, tt, :, ds(0, D)], ps[:, :, 0, :], vb_t[:, :, 0, :])
                nc.vector.tensor_add(
                    va[:, tt, :, ds(3 * D, D)], ps[:, :, 1, :],
                    vb_t[:, :, 1, :])
                # zero padded token rows (per-partition mask)
                nc.vector.tensor_scalar_mul(
                    va[:, tt, :, ds(0, D)], va[:, tt, :, ds(0, D)],
                    vm_t[:, tt:tt + 1])
                nc.vector.tensor_scalar_mul(
                    va[:, tt, :, ds(3 * D, D)], va[:, tt, :, ds(3 * D, D)],
                    vm_t[:, tt:tt + 1])
                yield

            def drain_gen(g):
                for _ in g:
                    pass

            # hp0's q waves; x streams in column halves so wave 0 starts
            # as soon as the first x DMA lands
            for wave in range(2):
                psw = psum.tile([128, 1024], F32, tag="s",
                                name="ps_q0", bufs=2)
                for ct in range(NCT):
                    for half in range(2):
                        tch = 2 * wave + half
                        nc.tensor.matmul(
                            psw[:, ts(half, 512)],
                            wq_t[:, ct, 0, :],
                            xsl(ct, 512 * tch, 512),
                            start=(ct == 0), stop=(ct == NCT - 1),
                            skip_group_check=True,
                        )
                for half in range(2):
                    tch = 2 * wave + half
                    nc.vector.tensor_scalar_add(
                        qT[0][:, ts(tch, 512)], psw[:, ts(half, 512)],
                        bqk_t[:, 0:1])
            # hp0's k waves
            for wave in range(2):
                psw = psum.tile([128, 1024], F32, tag="s",
                                name="ps_k0", bufs=2)
                for ct in range(NCT):
                    for half in range(2):
                        tch = 2 * wave + half
                        nc.tensor.matmul(
                            psw[:, ts(half, 512)],
                            wk_t[:, ct, 0, :],
                            xsl(ct, 512 * tch, 512),
                            start=(ct == 0), stop=(ct == NCT - 1),
                            skip_group_check=True,
                        )
                for half in range(2):
                    tch = 2 * wave + half
                    nc.vector.tensor_scalar_add(
                        kT[0][:, ts(tch, 512)], psw[:, ts(half, 512)],
                        bqk_t[:, 2:3])

            for tt in range(NVUP):
                drain_gen(v_chunk_gen(tt))

            # l-slot ones fills: ones * vmask broadcast into va cols
            # [64, 192) for every (tt, hp) -- replaces the omask DMAs.
            # Emitted after the upfront v drains so the DVE psum
            # evacuations aren't queued behind them.
            for tt in range(NKT):
                nc.vector.tensor_scalar_mul(
                    va[:, tt, :, ds(D, 2 * D)], warm[:, 0:2, :],
                    vm_t[:, tt:tt + 1])

            # deferred: v[NVUP..15], then hp1's q/k chunks interleaved so
            # each hp1 chunk's operands complete as early as possible
            deferred = []
            for tt in range(NVUP, NKT):
                deferred.append((("v", tt), v_chunk_gen(tt)))
            for tch in range(NCH):
                deferred.append(
                    (("k1", tch), qk_chunk_gen(wk_t, 2, kT, 1, tch)))
                deferred.append(
                    (("q1", tch), qk_chunk_gen(wq_t, 0, qT, 1, tch)))
            gens_done = set()

            def dstep():
                if not deferred:
                    return False
                name, g = deferred[0]
                try:
                    next(g)
                except StopIteration:
                    gens_done.add(name)
                    deferred.pop(0)
                return True

            def drequire(name):
                while name not in gens_done and deferred:
                    dstep()

            # ---- phase 2+3: attention per head-pair, then its proj ----
            attnT = [acts.tile([128, T], BF16, tag=f"aT{hp}", name=f"aT{hp}")
                     for hp in range(2)]

            def scores_pair(hp, ch, pp):
                """Score matmuls + causal bias + exp for k-tile pair pp,
                both heads. Returns (pp, [p2_h0, p2_h1], offs)."""
                q_lo = 512 * ch
                kt0 = 2 * pp
                pss = [psum.tile([128, 1024], F32, tag="s",
                                 name=f"ps_s{h}", bufs=2) for h in range(2)]
                p2s = [ppool.tile([128, 2, 512], BF16, tag="p",
                                  name=f"p{h}") for h in range(2)]
                offs = []
                for i in range(2):
                    kt = kt0 + i
                    diag = (kt >= 4 * ch)
                    off = 128 * (kt - 4 * ch) if diag else 0
                    offs.append(off)
                # hp0 (PE-paced): i-major bursts -- the two heads' 64-row
                # matmuls run concurrently in distinct PE row groups.
                # hp1 (ACT-paced): h-major -- head 0's scores, causal bias
                # and exp complete first so pv(h0) fills the PE bubble
                # while head 1's exp still runs.
                def mm_s(i, h):
                    off = offs[i]
                    n = 512 - off
                    nc.tensor.matmul(
                        pss[h][:, ds(512 * i + off, n)],
                        kT[hp][ds(64 * h, 64), ts(kt0 + i, 128)],
                        qT[hp][ds(64 * h, 64), ds(q_lo + off, n)],
                        start=True, stop=True,
                    )

                def tri_h(h):
                    # causal boundary: add -30k to the invalid triangle of
                    # the diagonal 128-blocks in PSUM, pre-exp (DVE)
                    for i in range(2):
                        if kt0 + i >= 4 * ch:
                            off = offs[i]
                            nc.vector.tensor_add(
                                pss[h][:, ds(512 * i + off, 128)],
                                pss[h][:, ds(512 * i + off, 128)],
                                tri_t[:])

                def exp_h(h):
                    # exp -> bf16 p, one grouped activation per head over
                    # both slabs (trimmed regions hold exp(stale) junk
                    # that the pv matmuls never read)
                    nc.scalar.activation(
                        p2s[h][:, :, :], pss[h][:, :], EXP,
                        bias=0.0, scale=0.125)

                if hp == 0:
                    for i in range(2):
                        for h in range(2):
                            mm_s(i, h)
                    for h in range(2):
                        tri_h(h)
                    for h in range(2):
                        exp_h(h)
                else:
                    for h in range(2):
                        for i in range(2):
                            mm_s(i, h)
                        tri_h(h)
                        exp_h(h)
                return (pp, p2s, offs)

            def pv_l2(hp, ent, ps_h, first, last):
                """Fused bf16 pv+l matmuls for one k-tile pair: lhsT is
                [v|ones] (h=0) / [ones|v] (h=1), out [128, n] accumulates o
                on the head's attnT partitions and l on the other half."""
                pp, p2s, offs = ent
                ihs = ([(i, h) for i in range(2) for h in range(2)]
                       if hp == 0 else
                       [(i, h) for h in range(2) for i in range(2)])
                for i, h in ihs:
                    kt = 2 * pp + i
                    off = offs[i]
                    n = 512 - off
                    nc.tensor.matmul(
                        ps_h[h][:, ds(off, n)],
                        va[:, kt, hp, ds(128 * h, 128)],
                        p2s[h][:, i, ds(off, n)],
                        start=(first and i == 0),
                        stop=(last and i == 1),
                        skip_group_check=True,
                    )

            def proj_tile(tt, tail=False):
                psc = [psum.tile([128, 512], F32, tag="pj",
                                 name=f"ps_y{c}", bufs=2) for c in range(2)]
                for hp in range(2):
                    for cch in range(2):
                        nc.tensor.matmul(
                            psc[cch][:],
                            attnT[hp][:, ts(tt, 128)],
                            wp_t[:, hp, ts(cch, 512)],
                            start=(hp == 0), stop=(hp == 1),
                            skip_group_check=True,
                        )
                ysb = ev.tile([128, 1024], BF16, tag="y", name="ysb")
                if tail:
                    # tail drain: split the copies so the ACT and DVE
                    # chains run in parallel
                    nc.vector.tensor_copy(ysb[:, 0:512], psc[0][:])
                    nc.scalar.copy(ysb[:, 512:1024], psc[1][:])
                else:
                    nc.vector.tensor_copy(ysb[:, 0:512], psc[0][:])
                    nc.vector.tensor_copy(ysb[:, 512:1024], psc[1][:])
                nc.sync.dma_start(y[ts(tt, 128), :], ysb[:])

            proj_q = []
            norm_q = []

            def normalize(hp, ch, osb):
                """Lazy normalize tail: attn^T = o / l from the SBUF (or,
                for the final chunk, PSUM) copies. The 1/l partition-shift
                DMA moves l to o's 64-partition half. For hp1 it also
                queues the chunk's proj tiles (emission strictly after the
                attnT writes)."""
                q_lo = 512 * ch
                rt0 = ev.tile([128, 512], F32, tag="rt0", name="rt0")
                rt1 = ev.tile([128, 512], F32, tag="rt1", name="rt1")
                rec = ev.tile([128, 512], F32, tag="rec", name="rec")
                nc.vector.reciprocal_approx_fast(rt0[:], osb[0][:])
                nc.vector.reciprocal_approx_fast(rt1[:], osb[1][:])
                # partition shift via quadrant-local shuffle with +-64
                # shifted AP bases (identity mask)
                nc.vector.stream_shuffle(
                    rec[ds(0, 64), :], rt0[ds(64, 64), :], list(range(32)))
                nc.vector.stream_shuffle(
                    rec[ds(64, 64), :], rt1[ds(0, 64), :], list(range(32)))
                from concourse.bass_primitives import MemorySpace
                mul_eng = (nc.gpsimd
                           if osb[0][:].space == MemorySpace.SBUF
                           else nc.vector)
                mul_eng.tensor_mul(
                    attnT[hp][ds(0, 64), ds(q_lo, 512)],
                    osb[0][ds(0, 64), :], rec[ds(0, 64), :])
                mul_eng.tensor_mul(
                    attnT[hp][ds(64, 64), ds(q_lo, 512)],
                    osb[1][ds(64, 64), :], rec[ds(64, 64), :])
                if hp == 1:
                    for tt in range(4 * ch, 4 * ch + 4):
                        proj_q.append(tt)

            # global software pipeline over all (hp, ch, pair): scores of
            # the next pair issue before the previous pair's pv, across
            # chunk boundaries; normalize tails and proj tiles drain lazily
            # one-per-slot so no engine sees a burst at a boundary.
            sched = []
            for hp in range(2):
                chorder = range(NCH)
                for ch in chorder:
                    npair = 2 * (ch + 1)
                    # diagonal pairs first: their causal-bias DVE hop
                    # hides behind the remaining pairs' scores
                    order = (list(range(2 * ch, npair)) +
                             list(range(2 * ch)))
                    for j, pp in enumerate(order):
                        sched.append((hp, ch, pp, j == 0, j == npair - 1))

            pend = []
            ps_h = None

            def pop_pend():
                (hp, ch, ent, fi, la, psh) = pend.pop(0)
                # va tiles for this pair must be complete before pv emission
                kt1 = 2 * ent[0] + 1
                if hp == 0 and kt1 >= NVUP:
                    drequire(("v", kt1))
                pv_l2(hp, ent, psh, fi, la)
                if la:
                    final = (hp == 1 and ch == NCH - 1)
                    if final:
                        normalize(hp, ch, psh)
                    else:
                        # copies free the o/l banks for the next chunk;
                        # the rest of the tail runs lazily off-path
                        osb = [ev.tile([128, 512], F32, tag=f"osb{h}",
                                       name=f"osb{h}") for h in range(2)]
                        nc.vector.tensor_copy(osb[0][:], psh[0][:])
                        nc.vector.tensor_copy(osb[1][:], psh[1][:])
                        norm_q.append((hp, ch, osb))

            for (hp, ch, pp, first, last) in sched:
                if first:
                    if hp == 1:
                        # the small deferred leftover drains inside hp1's
                        # early (otherwise sparse) slots; these requires
                        # are the correctness guard for this chunk's q/k
                        drequire(("k1", ch))
                        drequire(("q1", ch))
                        if ch == 2:
                            drequire(("v", NKT - 1))
                    ps_h = [psum.tile([128, 512], F32, tag="o",
                                      name="ps_h0"),
                            psum.tile([128, 512], F32, tag="l",
                                      name="ps_h1")]
                final = (hp == 1 and ch == NCH - 1)
                pend.append((hp, ch, scores_pair(hp, ch, pp), first, last,
                             ps_h))
                if hp == 1:
                    # ACT-paced phase: dummies placed between the scores
                    # and the exp-gated pv soak up the PE wait so the
                    # activity monitor doesn't re-throttle the clock
                    dps = psum.tile([128, 512], F32, tag="pj",
                                    name="ps_warm2", bufs=2)
                    for _ in range(2):
                        nc.tensor.matmul(dps[:], warm[:, 0, :],
                                         warm[:, 1:5, :], start=True,
                                         stop=True, skip_group_check=True)
                while len(pend) > 1:
                    pop_pend()
                # paced background work: deferred qkv, one lazy normalize
                # tail, one proj tile (reserved during the final chunk so
                # they overlap the tail normalize chain)
                steps = 8 if hp == 0 else 4
                while steps > 0 and dstep():
                    steps -= 1
                if norm_q:
                    normalize(*norm_q.pop(0))
                if proj_q:
                    proj_tile(proj_q.pop(0))
            while pend:
                pop_pend()
            while norm_q:
                normalize(*norm_q.pop(0))
            while proj_q:
                proj_tile(proj_q.pop(0), tail=True)

    nc.compile()
    return nc


def shard_inputs(x, key_padding_mask, Wqkv, bqkv, Wproj, bproj):
    bf = ml_dtypes.bfloat16
    # trib[p, c] = 0 where c >= p (valid causal), -30000 where c < p
    trib = np.where(np.arange(128)[:, None] <= np.arange(128)[None, :],
                    0.0, NEG).astype(np.float32)

    def wslice(w):
        # [C, CS] -> [128, ct, ot, 128]
        return np.ascontiguousarray(
            np.asarray(w).T.reshape(NCT, 128, 2, 128).transpose(1, 0, 2, 3)
        ).astype(bf)

    in_maps = []
    for core in range(NCORE):
        b, g = core // HPC, core % HPC
        qs = slice(CS * g, CS * g + CS)
        ks = slice(C + CS * g, C + CS * g + CS)
        vs = slice(2 * C + CS * g, 2 * C + CS * g + CS)
        km = np.asarray(key_padding_mask[b], np.float32).reshape(NKT, 128)
        bq = np.asarray(bqkv[qs]).reshape(2, 128).T
        bk = np.asarray(bqkv[ks]).reshape(2, 128).T
        xc = np.asarray(x[b]).T.reshape(NCT, 128, 2, 1024)  # [ct,p,half,c]
        in_maps.append({
            "xT": np.ascontiguousarray(xc.transpose(1, 2, 0, 3)).astype(bf),
            "wq": wslice(Wqkv[qs]),
            "wk": wslice(Wqkv[ks]),
            "wv": wslice(Wqkv[vs]),
            "wp": np.ascontiguousarray(
                np.asarray(Wproj[:, CS * g:CS * g + CS]).T.reshape(
                    2, 128, 1024).transpose(1, 0, 2)).astype(bf),
            "bqk": np.ascontiguousarray(
                np.concatenate([bq, bk], axis=1)).astype(np.float32),
            "vbias": np.ascontiguousarray(np.broadcast_to(
                np.asarray(bqkv[vs]).reshape(2, 2, D), (128, 2, 2, D))
            ).astype(np.float32),
            "vmask": np.ascontiguousarray(km.T),
            "trib": trib,
        })
    return in_maps


_NC_CACHE = None


def kernel(x, key_padding_mask, Wqkv, bqkv, Wproj, bproj):
    global _NC_CACHE
    if _NC_CACHE is None:
        _NC_CACHE = build_nc()
    nc = _NC_CACHE
    in_maps = shard_inputs(x, key_padding_mask, Wqkv, bqkv, Wproj, bproj)
    res = run_bass_kernel_spmd(nc, in_maps, list(range(NCORE)))
    if not all(np.isfinite(np.asarray(r["y"], dtype=np.float32)).all()
               for r in res.results):
        # very rare first-execution flake: retry once
        res = run_bass_kernel_spmd(nc, in_maps, list(range(NCORE)))
    out = np.empty((B, T, C), np.float32)
    for b in range(B):
        acc = np.zeros((T, C), np.float64)
        for g in range(HPC):
            acc += np.asarray(res.results[4 * b + g]["y"], dtype=np.float64)
        out[b] = (acc + np.asarray(bproj)).astype(np.float32)
    return out


# revision 29
# speedup vs baseline: 1.0438x; 1.0438x over previous
"""Causal self-attention Trainium2 kernel.

B=2, T=2048, C=1024, H=16, D=64. 8 NeuronCores: core = b*4 + head_group,
data parallel over batch (b = core//4), tensor parallel over heads
(4 heads per core). Each core computes its heads' qkv projection,
causal+key-masked attention, and a partial output projection over its
256 input channels; the host sums the per-core/per-head-pair partials
per batch element and adds the proj bias.

All on-device layouts are transposed so softmax works per-partition:
  xT   [C, T]      q^T/k^T [2*64, T] per head-pair (partition = head dim)
  s^T  [k, q]      out^T[d, q]  col-tiled p@v; lhsT of the proj matmul
The softmax denominator l is fused into the pv matmul via [v|ones] /
[ones|v] stationary operands; key padding is data (masked ones in the
l slots + masked v rows). The causal boundary is a -30k DVE bias add on
the PSUM diagonal blocks before exp.

v2 layout notes (DMA issue on the sync engine costs ~0.7us per DMA
regardless of size, so inputs are host-packed into one fused DRAM
tensor per logical matrix and moved with ~12 big DMAs; the l-slot ones
masks are generated on-device from vmask instead of DMA'd):
  x    [128, half, ct, 1024]   w{q,k,v} [128, ct, ot, 128]
  wp   [128, hp, 1024]         va [128, tt, hp, (vh0|ones|ones|vh1)]
"""

import sys

sys.path.insert(0, "/opt/trn_rl_repo")

import numpy as np
import ml_dtypes

import concourse.bass as bass
import concourse.mybir as mybir
import concourse.tile as tile
from concourse import bacc
from concourse.bass import ts, ds
from concourse.bass_utils import run_bass_kernel_spmd

B, T, C, H = 2, 2048, 1024, 16
D = C // H            # 64
HPC = 4               # heads per core
CS = HPC * D          # 256 channel slice per core
NCORE = 8
NKT = T // 128        # 16 k-tiles
NPAIR = NKT // 2      # 8 k-tile pairs
NCH = T // 512        # 4 q-chunks
NCT = C // 128        # 8 contraction tiles
F32 = mybir.dt.float32
BF16 = mybir.dt.bfloat16
EXP = mybir.ActivationFunctionType.Exp

NEG = -30000.0
NVUP = 4              # v chunks drained before attention starts


def build_nc():
    nc = bacc.Bacc("TRN2", target_bir_lowering=False, debug=False,
                   num_devices=NCORE)

    xT = nc.dram_tensor("xT", [128, 2, NCT, 1024], BF16, kind="ExternalInput")
    wq = nc.dram_tensor("wq", [128, NCT, 2, 128], BF16, kind="ExternalInput")
    wk = nc.dram_tensor("wk", [128, NCT, 2, 128], BF16, kind="ExternalInput")
    wv = nc.dram_tensor("wv", [128, NCT, 2, 128], BF16, kind="ExternalInput")
    wp = nc.dram_tensor("wp", [128, 2, 1024], BF16, kind="ExternalInput")
    bqk = nc.dram_tensor("bqk", [128, 4], F32, kind="ExternalInput")
    vbias = nc.dram_tensor("vbias", [128, 2, 2, D], F32, kind="ExternalInput")
    vmask = nc.dram_tensor("vmask", [128, NKT], F32, kind="ExternalInput")
    # trib[p, c] = 0 where c >= p (valid causal), -30000 where c < p;
    # added to the PSUM diagonal score blocks before exp.
    trib = nc.dram_tensor("trib", [128, 128], F32, kind="ExternalInput")
    y = nc.dram_tensor("y", [T, C], BF16, kind="ExternalOutput")

    with tile.TileContext(nc) as tc:
        with (
            tc.tile_pool(name="const", bufs=1) as const,
            tc.tile_pool(name="acts", bufs=1) as acts,
            tc.tile_pool(name="p", bufs=8) as ppool,
            tc.tile_pool(name="ev", bufs=4) as ev,
            tc.tile_pool(name="psum", bufs=1, space="PSUM") as psum,
        ):
            x_t = const.tile([128, 2, NCT, 1024], BF16, tag="x")
            wq_t = const.tile([128, NCT, 2, 128], BF16, tag="wq")
            wk_t = const.tile([128, NCT, 2, 128], BF16, tag="wk")
            wv_t = const.tile([128, NCT, 2, 128], BF16, tag="wv")
            wp_t = const.tile([128, 2, 1024], BF16, tag="wp")
            bqk_t = const.tile([128, 4], F32, tag="bqk")
            vb_t = const.tile([128, 2, 2, D], F32, tag="vb")
            vm_t = const.tile([128, NKT], F32, tag="vm")
            tri_t = const.tile([128, 128], F32, tag="tri")
            # warm-up / ones tile: [128, 5, 128] of 1.0
            warm = const.tile([128, 5, 128], BF16, tag="warm")

            # PE warm-up: dummy matmuls keep the HAM clock warming while
            # the input DMAs stream in; warm also acts as the ones source
            # for the l-slot fills.
            nc.vector.memset(warm[:], 1.0)
            wps = psum.tile([128, 512], F32, tag="pj", name="ps_warm", bufs=2)
            for i in range(10):
                nc.tensor.matmul(wps[:], warm[:, 0, :], warm[:, 1:5, :],
                                 start=True, stop=True, skip_group_check=True)

            # ---- input DMAs: few, big, ordered by first use ----
            nc.sync.dma_start(x_t[:, 0, 0:2, :], xT[:, 0, 0:2, :])
            nc.sync.dma_start(wq_t[:], wq[:])
            for c0 in range(2, NCT, 2):
                nc.sync.dma_start(x_t[:, 0, c0:c0 + 2, :],
                                  xT[:, 0, c0:c0 + 2, :])
            nc.sync.dma_start(wk_t[:], wk[:])
            nc.sync.dma_start(bqk_t[:], bqk[:])
            nc.sync.dma_start(wv_t[:], wv[:])
            nc.sync.dma_start(vb_t[:], vbias[:])
            nc.sync.dma_start(vm_t[:], vmask[:])
            nc.sync.dma_start(tri_t[:], trib[:])
            nc.sync.dma_start(x_t[:, 1, 0:4, :], xT[:, 1, 0:4, :])
            nc.sync.dma_start(x_t[:, 1, 4:8, :], xT[:, 1, 4:8, :])
            nc.sync.dma_start(wp_t[:], wp[:])

            # ---- phase 1: q^T/k^T/v; hp1's q/k and most of v are deferred
            # into attention-hp0's PE stream via generators ----
            qT = [acts.tile([128, T], BF16, tag=f"qT{hp}", name=f"qT{hp}")
                  for hp in range(2)]
            kT = [acts.tile([128, T], BF16, tag=f"kT{hp}", name=f"kT{hp}")
                  for hp in range(2)]
            # va [128 tok, tt, hp, 256]: per (tt, hp) the 256 cols are
            # [v_h0 | ones | ones | v_h1]; head h's 128-wide lhsT slice
            # [128h, 128h+128) is [v|ones] (h=0) / [ones|v] (h=1), so the
            # fused pv+l matmul writes o exactly on the partitions attnT
            # needs (o rows 0-63 for h=0, 64-127 for h=1; the softmax
            # denominator l lands on the other half). The ones slots are
            # vmask-masked so padded keys drop out of the denominator.
            va = acts.tile([128, NKT, 2, 4 * D], BF16, tag="va")

            def xsl(ct, lo, n):
                """x columns [lo, lo+n) of contraction tile ct."""
                return x_t[:, lo // 1024, ct, ds(lo % 1024, n)]

            def qk_chunk_gen(w_t, bcol, dst, ot, tch):
                ps = psum.tile([128, 512], F32, tag="pj", name="ps_qk",
                               bufs=2)
                for ct in range(NCT):
                    nc.tensor.matmul(
                        ps[:],
                        w_t[:, ct, ot, :],
                        xsl(ct, 512 * tch, 512),
                        start=(ct == 0), stop=(ct == NCT - 1),
                        skip_group_check=True,
                    )
                    yield
                nc.vector.tensor_scalar_add(
                    dst[ot][:, ts(tch, 512)], ps[:], bqk_t[:, bcol + ot:bcol + ot + 1])
                yield

            def v_chunk_gen(tt):
                ps = psum.tile([128, 2, 2, D], F32, tag="pj", name="ps_v",
                               bufs=2)
                for ct in range(NCT):
                    nc.tensor.matmul(
                        ps[:],
                        xsl(ct, 128 * tt, 128),
                        wv_t[:, ct, :, :],
                        start=(ct == 0), stop=(ct == NCT - 1),
                        skip_group_check=True,
                    )
                    yield
                # psum [128, (hp, h, d)]; h=0 -> cols 0:64, h=1 -> 192:256
                nc.vector.tensor_add(
                    va[:, tt, :, ds(0, D)], ps[:, :, 0, :], vb_t[:, :, 0, :])
                nc.vector.tensor_add(
                    va[:, tt, :, ds(3 * D, D)], ps[:, :, 1, :],
                    vb_t[:, :, 1, :])
                # zero padded token rows (per-partition mask)
                nc.vector.tensor_scalar_mul(
                    va[:, tt, :, ds(0, D)], va[:, tt, :, ds(0, D)],
                    vm_t[:, tt:tt + 1])
                nc.vector.tensor_scalar_mul(
                    va[:, tt, :, ds(3 * D, D)], va[:, tt, :, ds(3 * D, D)],
                    vm_t[:, tt:tt + 1])
                yield

            def drain_gen(g):
                for _ in g:
                    pass

            # hp0's q waves; x streams in column halves so wave 0 starts
            # as soon as the first x DMA lands
            for wave in range(2):
                psw = psum.tile([128, 1024], F32, tag="s",
                                name="ps_q0", bufs=2)
                for ct in range(NCT):
                    for half in range(2):
                        tch = 2 * wave + half
                        nc.tensor.matmul(
                            psw[:, ts(half, 512)],
                            wq_t[:, ct, 0, :],
                            xsl(ct, 512 * tch, 512),
                            start=(ct == 0), stop=(ct == NCT - 1),
                            skip_group_check=True,
                        )
                for half in range(2):
                    tch = 2 * wave + half
                    nc.vector.tensor_scalar_add(
                        qT[0][:, ts(tch, 512)], psw[:, ts(half, 512)],
                        bqk_t[:, 0:1])
            # hp0's k waves
            for wave in range(2):
                psw = psum.tile([128, 1024], F32, tag="s",
                                name="ps_k0", bufs=2)
                for ct in range(NCT):
                    for half in range(2):
                        tch = 2 * wave + half
                        nc.tensor.matmul(
                            psw[:, ts(half, 512)],
                            wk_t[:, ct, 0, :],
                            xsl(ct, 512 * tch, 512),
                            start=(ct == 0), stop=(ct == NCT - 1),
                            skip_group_check=True,
                        )
                for half in range(2):
                    tch = 2 * wave + half
                    nc.vector.tensor_scalar_add(
                        kT[0][:, ts(tch, 512)], psw[:, ts(half, 512)],
                        bqk_t[:, 2:3])

            for tt in range(NVUP):
                drain_gen(v_chunk_gen(tt))

            # l-slot ones fills: ones * vmask broadcast into va cols
            # [64, 192) for every (tt, hp) -- replaces the omask DMAs.
            # Emitted after the upfront v drains so the DVE psum
            # evacuations aren't queued behind them.
            for tt in range(NKT):
                nc.vector.tensor_scalar_mul(
                    va[:, tt, :, ds(D, 2 * D)], warm[:, 0:2, :],
                    vm_t[:, tt:tt + 1])

            # deferred: v[NVUP..15], then hp1's q/k chunks interleaved so
            # each hp1 chunk's operands complete as early as possible
            deferred = []
            for tt in range(NVUP, NKT):
                deferred.append((("v", tt), v_chunk_gen(tt)))
            for tch in range(NCH):
                deferred.append(
                    (("k1", tch), qk_chunk_gen(wk_t, 2, kT, 1, tch)))
                deferred.append(
                    (("q1", tch), qk_chunk_gen(wq_t, 0, qT, 1, tch)))
            gens_done = set()

            def dstep():
                if not deferred:
                    return False
                name, g = deferred[0]
                try:
                    next(g)
                except StopIteration:
                    gens_done.add(name)
                    deferred.pop(0)
                return True

            def drequire(name):
                while name not in gens_done and deferred:
                    dstep()

            # ---- phase 2+3: attention per head-pair, then its proj ----
            attnT = [acts.tile([128, T], BF16, tag=f"aT{hp}", name=f"aT{hp}")
                     for hp in range(2)]

            def scores_pair(hp, ch, pp):
                """Score matmuls + causal bias + exp for k-tile pair pp,
                both heads. Returns (pp, [p2_h0, p2_h1], offs)."""
                q_lo = 512 * ch
                kt0 = 2 * pp
                pss = [psum.tile([128, 1024], F32, tag="s",
                                 name=f"ps_s{h}", bufs=2) for h in range(2)]
                p2s = [ppool.tile([128, 2, 512], BF16, tag="p",
                                  name=f"p{h}") for h in range(2)]
                offs = []
                for i in range(2):
                    kt = kt0 + i
                    diag = (kt >= 4 * ch)
                    off = 128 * (kt - 4 * ch) if diag else 0
                    offs.append(off)
                # hp0 (PE-paced): i-major bursts -- the two heads' 64-row
                # matmuls run concurrently in distinct PE row groups.
                # hp1 (ACT-paced): h-major -- head 0's scores, causal bias
                # and exp complete first so pv(h0) fills the PE bubble
                # while head 1's exp still runs.
                def mm_s(i, h):
                    off = offs[i]
                    n = 512 - off
                    nc.tensor.matmul(
                        pss[h][:, ds(512 * i + off, n)],
                        kT[hp][ds(64 * h, 64), ts(kt0 + i, 128)],
                        qT[hp][ds(64 * h, 64), ds(q_lo + off, n)],
                        start=True, stop=True,
                    )

                def tri_h(h):
                    # causal boundary: add -30k to the invalid triangle of
                    # the diagonal 128-blocks in PSUM, pre-exp (DVE)
                    for i in range(2):
                        if kt0 + i >= 4 * ch:
                            off = offs[i]
                            nc.vector.tensor_add(
                                pss[h][:, ds(512 * i + off, 128)],
                                pss[h][:, ds(512 * i + off, 128)],
                                tri_t[:])

                def exp_h(h):
                    # exp -> bf16 p, one grouped activation per head over
                    # both slabs (trimmed regions hold exp(stale) junk
                    # that the pv matmuls never read)
                    nc.scalar.activation(
                        p2s[h][:, :, :], pss[h][:, :], EXP,
                        bias=0.0, scale=0.125)

                if hp == 0:
                    for i in range(2):
                        for h in range(2):
                            mm_s(i, h)
                    for h in range(2):
                        tri_h(h)
                    for h in range(2):
                        exp_h(h)
                else:
                    for h in range(2):
                        for i in range(2):
                            mm_s(i, h)
                        tri_h(h)
                        exp_h(h)
                return (pp, p2s, offs)

            def pv_l2(hp, ent, ps_h, first, last):
                """Fused bf16 pv+l matmuls for one k-tile pair: lhsT is
                [v|ones] (h=0) / [ones|v] (h=1), out [128, n] accumulates o
                on the head's attnT partitions and l on the other half."""
                pp, p2s, offs = ent
                ihs = ([(i, h) for i in range(2) for h in range(2)]
                       if hp == 0 else
                       [(i, h) for h in range(2) for i in range(2)])
                for i, h in ihs:
                    kt = 2 * pp + i
                    off = offs[i]
                    n = 512 - off
                    nc.tensor.matmul(
                        ps_h[h][:, ds(off, n)],
                        va[:, kt, hp, ds(128 * h, 128)],
                        p2s[h][:, i, ds(off, n)],
                        start=(first and i == 0),
                        stop=(last and i == 1),
                        skip_group_check=True,
                    )

            def proj_tile(tt, tail=False):
                psc = [psum.tile([128, 512], F32, tag="pj",
                                 name=f"ps_y{c}", bufs=2) for c in range(2)]
                for hp in range(2):
                    for cch in range(2):
                        nc.tensor.matmul(
                            psc[cch][:],
                            attnT[hp][:, ts(tt, 128)],
                            wp_t[:, hp, ts(cch, 512)],
                            start=(hp == 0), stop=(hp == 1),
                            skip_group_check=True,
                        )
                ysb = ev.tile([128, 1024], BF16, tag="y", name="ysb")
                if tail:
                    # tail drain: split the copies so the ACT and DVE
                    # chains run in parallel
                    nc.vector.tensor_copy(ysb[:, 0:512], psc[0][:])
                    nc.scalar.copy(ysb[:, 512:1024], psc[1][:])
                else:
                    nc.vector.tensor_copy(ysb[:, 0:512], psc[0][:])
                    nc.vector.tensor_copy(ysb[:, 512:1024], psc[1][:])
                nc.sync.dma_start(y[ts(tt, 128), :], ysb[:])

            proj_q = []
            norm_q = []

            def normalize(hp, ch, osb):
                """Lazy normalize tail: attn^T = o / l from the SBUF (or,
                for the final chunk, PSUM) copies. The 1/l partition-shift
                DMA moves l to o's 64-partition half. For hp1 it also
                queues the chunk's proj tiles (emission strictly after the
                attnT writes)."""
                q_lo = 512 * ch
                rt0 = ev.tile([128, 512], F32, tag="rt0", name="rt0")
                rt1 = ev.tile([128, 512], F32, tag="rt1", name="rt1")
                rec = ev.tile([128, 512], F32, tag="rec", name="rec")
                from concourse.bass_primitives import MemorySpace
                final = osb[0][:].space == MemorySpace.PSUM
                nc.vector.reciprocal_approx_fast(rt0[:], osb[0][:])
                nc.vector.reciprocal_approx_fast(rt1[:], osb[1][:])
                if final:
                    # tail: the DVE is idle and DMA latency is exposed, so
                    # shift 1/l across partitions with a quadrant-local
                    # shuffle (identity mask, +-64 shifted AP bases)
                    nc.vector.stream_shuffle(
                        rec[ds(0, 64), :], rt0[ds(64, 64), :],
                        list(range(32)))
                    nc.vector.stream_shuffle(
                        rec[ds(64, 64), :], rt1[ds(0, 64), :],
                        list(range(32)))
                else:
                    nc.sync.dma_start(rec[ds(0, 64), :], rt0[ds(64, 64), :])
                    nc.sync.dma_start(rec[ds(64, 64), :], rt1[ds(0, 64), :])
                from concourse.bass_primitives import MemorySpace
                mul_eng = (nc.gpsimd
                           if osb[0][:].space == MemorySpace.SBUF
                           else nc.vector)
                mul_eng.tensor_mul(
                    attnT[hp][ds(0, 64), ds(q_lo, 512)],
                    osb[0][ds(0, 64), :], rec[ds(0, 64), :])
                mul_eng.tensor_mul(
                    attnT[hp][ds(64, 64), ds(q_lo, 512)],
                    osb[1][ds(64, 64), :], rec[ds(64, 64), :])
                if hp == 1:
                    for tt in range(4 * ch, 4 * ch + 4):
                        proj_q.append(tt)

            # global software pipeline over all (hp, ch, pair): scores of
            # the next pair issue before the previous pair's pv, across
            # chunk boundaries; normalize tails and proj tiles drain lazily
            # one-per-slot so no engine sees a burst at a boundary.
            sched = []
            for hp in range(2):
                chorder = range(NCH)
                for ch in chorder:
                    npair = 2 * (ch + 1)
                    # diagonal pairs first: their causal-bias DVE hop
                    # hides behind the remaining pairs' scores
                    order = (list(range(2 * ch, npair)) +
                             list(range(2 * ch)))
                    for j, pp in enumerate(order):
                        sched.append((hp, ch, pp, j == 0, j == npair - 1))

            pend = []
            ps_h = None

            def pop_pend():
                (hp, ch, ent, fi, la, psh) = pend.pop(0)
                # va tiles for this pair must be complete before pv emission
                kt1 = 2 * ent[0] + 1
                if hp == 0 and kt1 >= NVUP:
                    drequire(("v", kt1))
                pv_l2(hp, ent, psh, fi, la)
                if la:
                    final = (hp == 1 and ch == NCH - 1)
                    if final:
                        normalize(hp, ch, psh)
                    else:
                        # copies free the o/l banks for the next chunk;
                        # the rest of the tail runs lazily off-path
                        osb = [ev.tile([128, 512], F32, tag=f"osb{h}",
                                       name=f"osb{h}") for h in range(2)]
                        nc.vector.tensor_copy(osb[0][:], psh[0][:])
                        nc.vector.tensor_copy(osb[1][:], psh[1][:])
                        norm_q.append((hp, ch, osb))

            for (hp, ch, pp, first, last) in sched:
                if first:
                    if hp == 1:
                        # the small deferred leftover drains inside hp1's
                        # early (otherwise sparse) slots; these requires
                        # are the correctness guard for this chunk's q/k
                        drequire(("k1", ch))
                        drequire(("q1", ch))
                        if ch == 2:
                            drequire(("v", NKT - 1))
                    ps_h = [psum.tile([128, 512], F32, tag="o",
                                      name="ps_h0"),
                            psum.tile([128, 512], F32, tag="l",
                                      name="ps_h1")]
                final = (hp == 1 and ch == NCH - 1)
                pend.append((hp, ch, scores_pair(hp, ch, pp), first, last,
                             ps_h))
                while len(pend) > 1:
                    pop_pend()
                # paced background work: deferred qkv, one lazy normalize
                # tail, one proj tile (reserved during the final chunk so
                # they overlap the tail normalize chain)
                steps = 8 if hp == 0 else 4
                while steps > 0 and dstep():
                    steps -= 1
                if norm_q:
                    normalize(*norm_q.pop(0))
                if proj_q:
                    proj_tile(proj_q.pop(0))
                elif hp == 1 and not deferred:
                    # sparse hp1 slot: dummy matmuls soak up the PE idle
                    # so the activity monitor doesn't re-throttle the
                    # clock to half rate
                    dps = psum.tile([128, 512], F32, tag="pj",
                                    name="ps_warm2", bufs=2)
                    for _ in range(4):
                        nc.tensor.matmul(dps[:], warm[:, 0, :],
                                         warm[:, 1:5, :], start=True,
                                         stop=True, skip_group_check=True)
            while pend:
                pop_pend()
            while norm_q:
                normalize(*norm_q.pop(0))
            while proj_q:
                proj_tile(proj_q.pop(0), tail=True)

    nc.compile()
    return nc


def shard_inputs(x, key_padding_mask, Wqkv, bqkv, Wproj, bproj):
    bf = ml_dtypes.bfloat16
    # trib[p, c] = 0 where c >= p (valid causal), -30000 where c < p
    trib = np.where(np.arange(128)[:, None] <= np.arange(128)[None, :],
                    0.0, NEG).astype(np.float32)

    def wslice(w):
        # [C, CS] -> [128, ct, ot, 128]
        return np.ascontiguousarray(
            np.asarray(w).T.reshape(NCT, 128, 2, 128).transpose(1, 0, 2, 3)
        ).astype(bf)

    in_maps = []
    for core in range(NCORE):
        b, g = core // HPC, core % HPC
        qs = slice(CS * g, CS * g + CS)
        ks = slice(C + CS * g, C + CS * g + CS)
        vs = slice(2 * C + CS * g, 2 * C + CS * g + CS)
        km = np.asarray(key_padding_mask[b], np.float32).reshape(NKT, 128)
        bq = np.asarray(bqkv[qs]).reshape(2, 128).T
        bk = np.asarray(bqkv[ks]).reshape(2, 128).T
        xc = np.asarray(x[b]).T.reshape(NCT, 128, 2, 1024)  # [ct,p,half,c]
        in_maps.append({
            "xT": np.ascontiguousarray(xc.transpose(1, 2, 0, 3)).astype(bf),
            "wq": wslice(Wqkv[qs]),
            "wk": wslice(Wqkv[ks]),
            "wv": wslice(Wqkv[vs]),
            "wp": np.ascontiguousarray(
                np.asarray(Wproj[:, CS * g:CS * g + CS]).T.reshape(
                    2, 128, 1024).transpose(1, 0, 2)).astype(bf),
            "bqk": np.ascontiguousarray(
                np.concatenate([bq, bk], axis=1)).astype(np.float32),
            "vbias": np.ascontiguousarray(np.broadcast_to(
                np.asarray(bqkv[vs]).reshape(2, 2, D), (128, 2, 2, D))
            ).astype(np.float32),
            "vmask": np.ascontiguousarray(km.T),
            "trib": trib,
        })
    return in_maps


_NC_CACHE = None


def kernel(x, key_padding_mask, Wqkv, bqkv, Wproj, bproj):
    global _NC_CACHE
    if _NC_CACHE is None:
        _NC_CACHE = build_nc()
    nc = _NC_CACHE
    in_maps = shard_inputs(x, key_padding_mask, Wqkv, bqkv, Wproj, bproj)
    res = run_bass_kernel_spmd(nc, in_maps, list(range(NCORE)))
    if not all(np.isfinite(np.asarray(r["y"], dtype=np.float32)).all()
               for r in res.results):
        # very rare first-execution flake: retry once
        res = run_bass_kernel_spmd(nc, in_maps, list(range(NCORE)))
    out = np.empty((B, T, C), np.float32)
    for b in range(B):
        acc = np.zeros((T, C), np.float64)
        for g in range(HPC):
            acc += np.asarray(res.results[4 * b + g]["y"], dtype=np.float64)
        out[b] = (acc + np.asarray(bproj)).astype(np.float32)
    return out
